# revision 6
# baseline (speedup 1.0000x reference)
"""GraphSAGE (2-layer, mean aggregation) Trainium2 kernel.

Sharding strategy (hardcoded): dst-range vertex partition. Edges are bucketed
on the host by destination node; core k owns nodes [k*12544, (k+1)*12544) and
receives exactly the edges targeting its nodes, so local segment sums are
complete — no all-reduce needed. Node features are replicated; the gather
table carries a constant-1 column so each gathered row contributes both the
feature aggregate and the degree count. An AllGather shares layer-1
activations between the two layers.

Edge layout ("degree-slotted"): within each 128-node destination block, the
i-th edge into node (block b, lane p) is placed at lane p of gather-tile i of
block b. One indirect DMA per tile gathers table[src] rows already aligned to
their destination lane, so a block's aggregate is a single vector
tensor_reduce over its tiles — no scatter, no sorting on device. Slots with
no edge gather an all-zero table row. Per-block tile counts come from the
actual degree distribution (the kernel is compiled per input shapes).
"""

import sys

sys.path.insert(0, "/opt/trn_rl_repo")

import numpy as np

N_NODES = 100000
N_EDGES = 6400000
F_IN, F_HID, F_OUT = 5, 5, 10
N_CORES = 8
P = 128
BLOCKS_PER_CORE = 98
NODES_PER_CORE = BLOCKS_PER_CORE * P  # 12544
N_PAD = N_CORES * NODES_PER_CORE  # 100352
ZERO_ROW = N_NODES  # gather row used by empty slots; kept all-zero
N_BLOCKS = N_CORES * BLOCKS_PER_CORE


def _build_nc(tiles_per_block, col_off, table_rows=N_PAD,
              blocks_per_core=BLOCKS_PER_CORE, f_in=F_IN, f_hid=F_HID,
              f_out=F_OUT, n_cores=N_CORES, zero_row=ZERO_ROW):
    import concourse.bacc as bacc
    import concourse.bass as bass
    import concourse.mybir as mybir
    import concourse.tile as tile

    f32 = mybir.dt.float32
    i32 = mybir.dt.int32
    C = blocks_per_core
    NT = int(col_off[-1])
    nodes_per_core = C * P
    g1 = f_in + 1   # gathered row width, layer 1
    g2 = f_hid + 1  # gathered row width, layer 2

    nc = bacc.Bacc("TRN2", target_bir_lowering=False)

    table_d = nc.dram_tensor("table", [table_rows, g1], f32, kind="ExternalInput")
    idx_d = nc.dram_tensor("idx_mat", [P, NT], i32, kind="ExternalInput")
    xown_d = nc.dram_tensor("x_own", [P, C * f_in], f32, kind="ExternalInput")
    wb1_d = nc.dram_tensor("wb1", [P, 2 * f_in * f_hid], f32, kind="ExternalInput")
    bb1_d = nc.dram_tensor("bb1", [P, f_hid], f32, kind="ExternalInput")
    wb2_d = nc.dram_tensor("wb2", [P, 2 * f_hid * f_out], f32, kind="ExternalInput")
    bb2_d = nc.dram_tensor("bb2", [P, f_out], f32, kind="ExternalInput")
    out_d = nc.dram_tensor("out", [P, C * f_out], f32, kind="ExternalOutput")

    h_own_d = nc.dram_tensor("h_own_b", [nodes_per_core, g2], f32)
    h_all_d = nc.dram_tensor("h_all_b", [n_cores * nodes_per_core, g2], f32)

    with tile.TileContext(nc) as tc:
        with (
            tc.tile_pool(name="big", bufs=1) as big,
            tc.tile_pool(name="mp", bufs=4) as mp,
            tc.tile_pool(name="misc", bufs=2) as misc,
        ):
            idx_t = big.tile([P, NT], i32, tag="idx")
            nc.sync.dma_start(out=idx_t[:], in_=idx_d[:])
            xown_t = big.tile([P, C * f_in], f32, tag="xo")
            nc.sync.dma_start(out=xown_t[:], in_=xown_d[:])
            wb1_t = big.tile([P, 2 * f_in * f_hid], f32, tag="w1")
            nc.sync.dma_start(out=wb1_t[:], in_=wb1_d[:])
            bb1_t = big.tile([P, f_hid], f32, tag="B1")
            nc.sync.dma_start(out=bb1_t[:], in_=bb1_d[:])
            wb2_t = big.tile([P, 2 * f_hid * f_out], f32, tag="w2")
            nc.sync.dma_start(out=wb2_t[:], in_=wb2_d[:])
            bb2_t = big.tile([P, f_out], f32, tag="B2")
            nc.sync.dma_start(out=bb2_t[:], in_=bb2_d[:])

            def edge_pass(src_table_d, gw, agg_t):
                for b in range(C):
                    t2 = int(tiles_per_block[b])
                    base = int(col_off[b])
                    buf = mp.tile([P, t2 * gw], f32, tag="buf")
                    for i in range(t2):
                        nc.gpsimd.indirect_dma_start(
                            out=buf[:, i * gw : (i + 1) * gw],
                            out_offset=None,
                            in_=src_table_d[:],
                            in_offset=bass.IndirectOffsetOnAxis(
                                ap=idx_t[:, base + i : base + i + 1], axis=0
                            ),
                        )
                    nc.vector.tensor_reduce(
                        out=agg_t[:, b * gw : (b + 1) * gw],
                        in_=buf[:, : t2 * gw].rearrange("p (t f) -> p f t", f=gw),
                        axis=mybir.AxisListType.X,
                        op=mybir.AluOpType.add,
                    )

            def dense(agg_t, gw, fi, fo, ownv, wb_t, bb_t, out_v):
                # out_v[p,c,j] = sigmoid( sum_f own[p,c,f]*W_self[f,j]
                #   + sum_f (agg/max(deg,1))[p,c,f]*W_neigh[f,j] + b[j] )
                aggv = agg_t[:].rearrange("p (c f) -> p c f", f=gw)
                deg_t = misc.tile([P, C], f32, tag="deg")
                nc.vector.tensor_scalar_max(deg_t[:], aggv[:, :, gw - 1], 1.0)
                rec_t = misc.tile([P, C], f32, tag="rec")
                nc.vector.reciprocal(rec_t[:], deg_t[:])
                mean_t = misc.tile([P, C * fi], f32, tag="mean")
                meanv = mean_t[:].rearrange("p (c f) -> p c f", f=fi)
                for f in range(fi):
                    nc.vector.tensor_tensor(
                        out=meanv[:, :, f], in0=aggv[:, :, f], in1=rec_t[:],
                        op=mybir.AluOpType.mult,
                    )
                for j in range(fo):
                    acc = misc.tile([P, C], f32, tag="acc")
                    nc.vector.tensor_tensor(
                        out=acc[:], in0=ownv[:, :, 0],
                        in1=wb_t[:, j : j + 1].to_broadcast([P, C]),
                        op=mybir.AluOpType.mult,
                    )
                    tmp = misc.tile([P, C], f32, tag="tmp")
                    for f in range(1, fi):
                        nc.vector.tensor_tensor(
                            out=tmp[:], in0=ownv[:, :, f],
                            in1=wb_t[:, f * fo + j : f * fo + j + 1].to_broadcast([P, C]),
                            op=mybir.AluOpType.mult,
                        )
                        nc.vector.tensor_tensor(
                            out=acc[:], in0=acc[:], in1=tmp[:],
                            op=mybir.AluOpType.add,
                        )
                    for f in range(fi):
                        w_off = fi * fo + f * fo + j
                        nc.vector.tensor_tensor(
                            out=tmp[:], in0=meanv[:, :, f],
                            in1=wb_t[:, w_off : w_off + 1].to_broadcast([P, C]),
                            op=mybir.AluOpType.mult,
                        )
                        nc.vector.tensor_tensor(
                            out=acc[:], in0=acc[:], in1=tmp[:],
                            op=mybir.AluOpType.add,
                        )
                    nc.vector.tensor_tensor(
                        out=acc[:], in0=acc[:],
                        in1=bb_t[:, j : j + 1].to_broadcast([P, C]),
                        op=mybir.AluOpType.add,
                    )
                    nc.scalar.activation(
                        out=out_v[:, :, j], in_=acc[:],
                        func=mybir.ActivationFunctionType.Sigmoid,
                    )

            # ---- layer 1 ----
            agg1_t = big.tile([P, C * g1], f32, tag="agg1")
            edge_pass(table_d, g1, agg1_t)
            h6_t = big.tile([P, C * g2], f32, tag="h6")
            h6v = h6_t[:].rearrange("p (c f) -> p c f", f=g2)
            dense(agg1_t, g1, f_in, f_hid,
                  xown_t[:].rearrange("p (c f) -> p c f", f=f_in),
                  wb1_t[:], bb1_t[:], h6v)
            nc.vector.memset(h6v[:, :, g2 - 1], 1.0)

            # share h: write own block, AllGather, zero the padding-gather row
            nc.sync.dma_start(
                out=h_own_d[:].rearrange("(c p) f -> p c f", p=P),
                in_=h6v,
            )
            nc.gpsimd.collective_compute(
                "AllGather",
                mybir.AluOpType.bypass,
                replica_groups=[list(range(n_cores))],
                ins=[h_own_d.ap().opt()],
                outs=[h_all_d.ap().opt()],
            )
            zrow = misc.tile([1, g2], f32, tag="z")
            nc.vector.memset(zrow[:], 0.0)
            nc.sync.dma_start(out=h_all_d[zero_row : zero_row + 1, :], in_=zrow[:])

            # ---- layer 2 ----
            agg2_t = big.tile([P, C * g2], f32, tag="agg2")
            edge_pass(h_all_d, g2, agg2_t)
            out_t = big.tile([P, C * f_out], f32, tag="out")
            outv = out_t[:].rearrange("p (c f) -> p c f", f=f_out)
            hown_v = h6_t[:].rearrange("p (c f) -> p c f", f=g2)[:, :, 0:f_hid]
            dense(agg2_t, g2, f_hid, f_out, hown_v, wb2_t[:], bb2_t[:], outv)
            nc.sync.dma_start(out=out_d[:], in_=out_t[:])

    nc.compile()
    return nc


def _host_prep(src, dst, n_pad=N_PAD, zero_row=ZERO_ROW,
               blocks_per_core=BLOCKS_PER_CORE, n_cores=N_CORES):
    """Degree-slotted layout: edge (rank i into node v) -> column col_off[b]+i,
    lane v&127 of core v//nodes_per_core, where b = (v % nodes_per_core) >> 7."""
    src = np.asarray(src).astype(np.int64)
    dst = np.asarray(dst).astype(np.int64)
    E = src.shape[0]
    nodes_per_core = blocks_per_core * P

    order = np.argsort(dst, kind="stable")
    src_s = src[order]
    dst_s = dst[order]
    deg = np.bincount(dst_s, minlength=n_pad)
    starts = np.zeros(n_pad + 1, np.int64)
    np.cumsum(deg, out=starts[1:])
    rank = np.arange(E, dtype=np.int64) - starts[dst_s]

    # per-block tile counts: max degree within block slot b across cores
    degmat = deg.reshape(n_cores, blocks_per_core, P)
    tiles_per_block = np.maximum(degmat.max(axis=(0, 2)), 1).astype(np.int64)
    col_off = np.zeros(blocks_per_core + 1, np.int64)
    np.cumsum(tiles_per_block, out=col_off[1:])
    NT = int(col_off[-1])

    core = dst_s // nodes_per_core
    rem = dst_s % nodes_per_core
    blk = rem >> 7
    lane = rem & 127
    col = col_off[blk] + rank

    idx_mats = np.full((n_cores, P, NT), zero_row, np.int32)
    idx_mats[core, lane, col] = src_s.astype(np.int32)
    return idx_mats, tiles_per_block, col_off


def kernel(x, src, dst, W_self1, W_neigh1, b1, W_self2, W_neigh2, b2):
    from concourse.bass_utils import run_bass_kernel_spmd

    x = np.asarray(x, np.float32)
    idx_mats, tiles_per_block, col_off = _host_prep(src, dst)
    nc = _build_nc(tiles_per_block, col_off)

    table = np.zeros((N_PAD, F_IN + 1), np.float32)
    table[:N_NODES, :F_IN] = x
    table[:N_NODES, F_IN] = 1.0

    def bcast(a):
        a = np.asarray(a, np.float32).reshape(1, -1)
        return np.broadcast_to(a, (P, a.shape[1])).copy()

    wb1 = bcast(np.concatenate([np.asarray(W_self1).ravel(), np.asarray(W_neigh1).ravel()]))
    wb2 = bcast(np.concatenate([np.asarray(W_self2).ravel(), np.asarray(W_neigh2).ravel()]))
    bb1 = bcast(b1)
    bb2 = bcast(b2)

    in_maps = []
    for k in range(N_CORES):
        base = k * NODES_PER_CORE
        xo = table[base : base + NODES_PER_CORE, :F_IN]  # rows (c*128+p)
        x_own = (
            xo.reshape(BLOCKS_PER_CORE, P, F_IN).transpose(1, 0, 2).reshape(P, -1).copy()
        )
        in_maps.append(
            {
                "table": table,
                "idx_mat": idx_mats[k],
                "x_own": x_own,
                "wb1": wb1,
                "bb1": bb1,
                "wb2": wb2,
                "bb2": bb2,
            }
        )

    res = run_bass_kernel_spmd(nc, in_maps, core_ids=list(range(N_CORES)))
    out = np.zeros((N_PAD, F_OUT), np.float32)
    for k in range(N_CORES):
        o = res.results[k]["out"]  # [P, C*F_OUT]
        o = o.reshape(P, BLOCKS_PER_CORE, F_OUT).transpose(1, 0, 2).reshape(-1, F_OUT)
        out[k * NODES_PER_CORE : (k + 1) * NODES_PER_CORE] = o
    return out[:N_NODES]


if __name__ == "__main__":
    print("module ok")
